# revision 1
# baseline (speedup 1.0000x reference)
"""BertSelfAttention on 8 Trainium2 NeuronCores (Bass/Tile).

Sharding: tensor-parallel over heads. 16 heads / 8 cores = 2 heads (128
head-dim columns) per core. Each core computes the Q/K/V projections for
its 128 output dims over all 4096 tokens, then attention for its 2 heads
over both batches, producing a [128, 4096] d-major slice of the output.
The host transposes hidden_states once, feeds every core the same
[1024, 4096] activation matrix plus its private weight slice, and
reassembles the full [2, 2048, 1024] output from the 8 slices.

Device-side layout choices (all driven by "the PE contracts over the
partition dim" + "softmax reductions are only cheap along the free dim is
false here -- we need them along keys"):
  - Q,K,V are produced d-major: [dim, token]. QK^T then yields scores
    TRANSPOSED, S^T[key, query], with keys on partitions.
  - exp() runs on ScalarE with the 1/sqrt(64) scale folded in.
  - V is PE-transposed to token-major tiles with a ones-column appended;
    the PV matmul (contracting over keys) then computes both the context
    AND the softmax denominator (row 64 of the psum) in one pass.
  - The denominator reciprocal is broadcast across partitions with a
    K=1 matmul against a ones row, then one DVE multiply normalizes.
  - All matmul operands are float32r (full-rate fp32 on the PE; plain
    fp32 runs at 1/4 rate). Measured end-to-end scale-relative absmax
    vs the fp32 reference: ~2.6e-4.
  - Emission order IS the per-engine schedule, so the whole kernel is
    software-pipelined by hand: batch-0 qb0's chunks are interleaved
    with the projection blocks they consume, and batch-1's projections
    and V-transposes ride as per-chunk fillers inside earlier attention
    blocks. A dummy-matmul warmup spans the initial DMA wait to lift
    the PE HAM clock gate before real work arrives.
"""

import numpy as np

import concourse.tile as tile
from concourse import bacc, mybir
from concourse.bass_utils import run_bass_kernel_spmd
from concourse.masks import make_identity

# Problem shape (hardcoded; harness contract)
B, S, H = 2, 2048, 1024
NUM_HEADS, DH = 16, 64
NCORES = 8
T = B * S                 # 4096 tokens total
D = H // NCORES           # 128 output dims per core (2 heads)
KC = H // 128             # 8 contraction chunks for projections
QB = 512                  # query-block width (one psum bank)
NQB = S // QB             # 4 query blocks per batch
NTB = T // QB             # 8 projection token-blocks
NKT = S // 128            # 16 key tiles per batch
KTC = 2                   # key tiles per exp chunk (psum-budget limited)
SCALE = 1.0 / float(np.sqrt(DH))

F32 = mybir.dt.float32
F32R = mybir.dt.float32r
EXP = mybir.ActivationFunctionType.Exp


def build(use_mask: bool, use_bias: bool, reps: int = 1):
    nc = bacc.Bacc("TRN2", target_bir_lowering=False)

    hT = nc.dram_tensor("hT", [H, T], F32, kind="ExternalInput")
    wq = nc.dram_tensor("wq", [H, D], F32, kind="ExternalInput")
    wk = nc.dram_tensor("wk", [H, D], F32, kind="ExternalInput")
    wv = nc.dram_tensor("wv", [H, D], F32, kind="ExternalInput")
    if use_bias:
        bq = nc.dram_tensor("bq", [D, 1], F32, kind="ExternalInput")
        bk = nc.dram_tensor("bk", [D, 1], F32, kind="ExternalInput")
        bv = nc.dram_tensor("bv", [D, 1], F32, kind="ExternalInput")
    if use_mask:
        # host pre-transposes to [128, B, NKT] so the DMA is contiguous
        mask = nc.dram_tensor("mask", [128, B, NKT], F32, kind="ExternalInput")
    out = nc.dram_tensor("out", [D, T], F32, kind="ExternalOutput")

    with tile.TileContext(nc) as tc:
        with (
            tc.tile_pool(name="consts", bufs=1) as consts,
            tc.tile_pool(name="qkv", bufs=1) as qkvp,
            tc.tile_pool(name="ht", bufs=2) as htp,
            tc.tile_pool(name="vtm", bufs=2) as vtmp,
            tc.tile_pool(name="e", bufs=10) as ep,
            tc.tile_pool(name="small", bufs=4) as smallp,
            tc.tile_pool(name="pp", bufs=2, space="PSUM") as pp,
            tc.tile_pool(name="qk", bufs=2, space="PSUM") as qkp,
            tc.tile_pool(name="pv", bufs=2, space="PSUM") as pvp,
        ):
            # ---- constants ----
            # DMA order matters: the sync queue drains in emission order and
            # the first K-projection needs only wk + the first hT block, so
            # wk is loaded first (it also feeds the PE warmup), the first hT
            # block is emitted next (from the driver), then wv/wq follow.
            w_sb = {}
            b_sb = {}
            w_dram = {"q": wq, "k": wk, "v": wv}
            for name in ("q", "k", "v"):
                w_sb[name] = consts.tile(
                    [128, KC, D], F32R, tag=f"w{name}", name=f"w{name}"
                )

            def load_w(name):
                nc.sync.dma_start(
                    out=w_sb[name][:],
                    in_=w_dram[name]
                    .rearrange("(kc p) d -> p kc d", p=128)
                    .bitcast(F32R),
                )

            load_w("q")
            load_w("k")
            load_w("v")
            if use_bias:
                for name, bt in (("q", bq), ("k", bk), ("v", bv)):
                    b_t = consts.tile([128, 1], F32, tag=f"b{name}")
                    nc.sync.dma_start(out=b_t[:], in_=bt[:])
                    b_sb[name] = b_t
            ident = consts.tile([128, 128], F32, tag="ident")
            make_identity(nc, ident[:])
            # ones constants (memset can't encode f32r; stage via f32 + copy)
            ones_st = consts.tile([128, DH], F32, tag="onesst")
            nc.vector.memset(ones_st[:], 1.0)
            # row at partition 64: stationary operand of the reciprocal-
            # broadcast matmuls (K=1 contraction at row 64, dst partitions 0-63)
            ones_row = consts.tile([65, DH], F32R, tag="onesrow")
            nc.vector.tensor_copy(ones_row[64:65, :], ones_st[64:65, :])
            if use_mask:
                mask_sb = consts.tile([128, B, NKT], F32, tag="mask")
                nc.sync.dma_start(out=mask_sb[:], in_=mask[:])

            # per-token-block tiles so attention dependencies are precise
            # (attention on batch 0 starts as soon as its blocks are done)
            Qts = [qkvp.tile([128, QB], F32R, tag=f"Qd{i}", name=f"Qd{i}") for i in range(NTB)]
            Kts = [qkvp.tile([128, QB], F32R, tag=f"Kd{i}", name=f"Kd{i}") for i in range(NTB)]
            Vts = [qkvp.tile([128, QB], F32, tag=f"Vd{i}", name=f"Vd{i}") for i in range(NTB)]

            hT_r = hT.rearrange("(kc p) t -> p kc t", p=128)

            def proj_load(tb):
                t0 = tb * QB
                ht_t = htp.tile([128, KC, QB], F32R, tag="ht", name="ht")
                nc.sync.dma_start(
                    out=ht_t[:], in_=hT_r[:, :, t0 : t0 + QB].bitcast(F32R)
                )
                return ht_t

            def proj_group(tb, ht_t, name):
                dest = {"q": Qts, "k": Kts, "v": Vts}[name][tb]
                ps = pp.tile([128, QB], F32, tag="pp", name="ps")
                for kc in range(KC):
                    nc.tensor.matmul(
                        ps[:],
                        w_sb[name][:, kc, :],
                        ht_t[:, kc, :],
                        start=(kc == 0),
                        stop=(kc == KC - 1),
                    )
                if use_bias:
                    nc.vector.tensor_scalar_add(dest[:], ps[:], b_sb[name][:])
                else:
                    nc.vector.tensor_copy(dest[:], ps[:])

            def project(tb):
                ht_t = proj_load(tb)
                for name in ("q", "k", "v"):
                    proj_group(tb, ht_t, name)

            # V token-major tiles per k-tile, 130 wide:
            # [h0 dims 0-63 | ones | h1 dims 65-128 | ones]; the ones
            # column makes the PV matmul also produce the softmax
            # denominator in psum row 64. One tile per k-tile so the PV
            # matmuls only depend on their own slice of V.
            vtms = {}

            def transpose_v(b, kt, pool, tag):
                g0 = b * S + kt * 128
                tbi, off = divmod(g0, QB)
                vt = vtmp.tile([128, 130], F32R, tag=f"vtm{kt}", name=f"vtm{kt}")
                nc.vector.tensor_copy(
                    vt[:, 64::65].rearrange("p (a o) -> p a o", o=1),
                    ones_st[:, 0:2].rearrange("p (a o) -> p a o", o=1),
                )
                tps = pool.tile([128, 128], F32, tag=tag, name="tps")
                nc.tensor.transpose(tps[:], Vts[tbi][:, off : off + 128], ident[:])
                nc.vector.tensor_copy(
                    vt[:].rearrange("p (g c) -> p g c", g=2)[:, :, 0:64],
                    tps.rearrange("p (g c) -> p g c", g=2),
                )
                vtms[(b, kt)] = vt

            # ---- explicit software-pipelined emission ----
            # Engines execute their instruction streams in emission order, so
            # the emission sequence IS the schedule. Batch-0 qb0 is emitted
            # chunk-by-chunk interleaved with the projections it waits on;
            # every later projection / V-transpose is a per-chunk "filler"
            # inside an earlier attention block so PE slack absorbs it and
            # ACT (the bottleneck) never starves.

            def attn_open(b, qb):
                # per head: one bank [65, QB] = context rows 0-63 +
                # softmax denominator row 64, one accumulation group
                ctx_ps = [
                    pvp.tile([65, QB], F32, tag="ctx", name=f"ctx{h}")
                    for h in range(2)
                ]
                return (b, qb, ctx_ps)

            def attn_qk(state, ktc):
                """Emit one chunk's QK^T matmuls; both heads adjacent so
                their disjoint PE row-groups (0-63 / 64-127) overlap.
                Returns the score psum tiles for the exp/PV part."""
                b, qb, _ = state
                q0 = b * S + qb * QB
                sps = [
                    qkp.tile([128, KTC, QB], F32, tag="sps", name=f"sps{h}")
                    for h in range(2)
                ]
                for j in range(KTC):
                    kt = ktc * KTC + j
                    tbi, off = divmod(b * S + kt * 128, QB)
                    for h in (0, 1):
                        nc.tensor.matmul(
                            sps[h][:, j, :],
                            Kts[tbi][h * 64 : (h + 1) * 64, off : off + 128],
                            Qts[q0 // QB][h * 64 : (h + 1) * 64, :],
                            start=True,
                            stop=True,
                        )
                return sps

            def attn_chunks(state, ktcs, fillers=(), mid_fillers=(), pre_qk=None):
                b, qb, ctx_ps = state
                q0 = b * S + qb * QB
                fillers = list(fillers)
                mid_fillers = list(mid_fillers)
                for ktc in ktcs:
                    if fillers:
                        fillers.pop(0)()
                    if pre_qk is not None and ktc == ktcs[0]:
                        sps = pre_qk
                    else:
                        sps = attn_qk(state, ktc)
                    ets = []
                    for h in (0, 1):
                        et = ep.tile([128, KTC, QB], F32R, tag="e", name=f"et{h}")
                        if use_mask:
                            for j in range(KTC):
                                kt = ktc * KTC + j
                                nc.scalar.activation(
                                    et[:, j, :],
                                    sps[h][:, j, :],
                                    EXP,
                                    bias=mask_sb[:, b, kt : kt + 1],
                                    scale=SCALE,
                                )
                        else:
                            nc.scalar.activation(et[:], sps[h][:], EXP, scale=SCALE)
                        ets.append(et)
                    if mid_fillers:
                        mid_fillers.pop(0)()
                    for j in range(KTC):
                        kt = ktc * KTC + j
                        for h in (0, 1):
                            nc.tensor.matmul(
                                ctx_ps[h][:],
                                vtms[(b, kt)][:, h * 65 : (h + 1) * 65],
                                ets[h][:, j, :],
                                start=(kt == 0),
                                stop=(kt == NKT - 1),
                            )
                # leftover fillers (if any) run after the chunks
                for f in fillers:
                    f()

            def attn_close(state):
                """Emit the reciprocals now (DVE, off the critical PE path)
                and return a thunk with the PE/DVE normalize tail. Deferring
                that thunk into the NEXT query block's first chunk (after
                its QK/exp, before its PV) removes the ~2us ACT stall at
                every block boundary: the next block's QKs reach the PE
                stream ahead of the broadcast matmuls."""
                b, qb, ctx_ps = state
                q0 = b * S + qb * QB
                recs = []
                for h in (0, 1):
                    rec = smallp.tile([65, QB], F32R, tag="rec", name=f"rec{h}")
                    with nc.allow_low_precision(reason="bcast operand"):
                        nc.vector.reciprocal(rec[64:65, :], ctx_ps[h][64:65, :])
                    recs.append(rec)

                def finish():
                    for h in (0, 1):
                        bc = pp.tile([64, QB], F32, tag="pp")
                        nc.tensor.matmul(
                            bc[:],
                            ones_row[64:65, :],
                            recs[h][64:65, :],
                            start=True,
                            stop=True,
                        )
                        # DVE TensorTensor cannot read two PSUM operands;
                        # stage the broadcast reciprocal through SBUF
                        bc_sb = smallp.tile([64, QB], F32, tag="bcsb")
                        nc.vector.tensor_copy(bc_sb[:], bc[:])
                        ot = smallp.tile([64, QB], F32, tag="ot")
                        nc.vector.tensor_mul(ot[:], ctx_ps[h][0:64, :], bc_sb[:])
                        nc.sync.dma_start(
                            out=out[h * 64 : (h + 1) * 64, q0 : q0 + QB],
                            in_=ot[:],
                        )

                return finish

            # --- the pipeline driver ---
            # PE warmup: the HAM clock gate keeps the PE at half rate until
            # it has been busy ~3.4us. The first hT block takes ~9us of DMA
            # before any real matmul can start, so burn that idle time on
            # dummy matmuls (weights are the first DMA to land) to enter the
            # attention pipeline at full clock.
            warm = qkp.tile([128, KTC, QB], F32, tag="sps", name="warm")
            for _ in range(12):
                nc.tensor.matmul(
                    warm[:, 0, :],
                    w_sb["q"][:, 0, :],
                    w_sb["q"][:, 0:4, :],
                    start=True,
                    stop=True,
                )

            hts = {}

            def load(tb):
                def f():
                    hts[tb] = proj_load(tb)

                return f

            def grp(tb, n):
                return lambda: proj_group(tb, hts[tb], n)

            def grp_halves(tb, n):
                """Split one projection group into two 4-kc emission halves
                (same psum accumulation bracket) so a filler never injects
                more than ~0.9us of PE work between attention chunks."""
                stash = {}

                def h1():
                    dest = {"q": Qts, "k": Kts, "v": Vts}[n][tb]
                    ps = pp.tile([128, QB], F32, tag="pp", name="ps")
                    stash["ps"], stash["dest"] = ps, dest
                    for kc in range(KC // 2):
                        nc.tensor.matmul(
                            ps[:],
                            w_sb[n][:, kc, :],
                            hts[tb][:, kc, :],
                            start=(kc == 0),
                            stop=False,
                        )

                def h2():
                    ps, dest = stash["ps"], stash["dest"]
                    for kc in range(KC // 2, KC):
                        nc.tensor.matmul(
                            ps[:],
                            w_sb[n][:, kc, :],
                            hts[tb][:, kc, :],
                            start=False,
                            stop=(kc == KC - 1),
                        )
                    if use_bias:
                        nc.vector.tensor_scalar_add(dest[:], ps[:], b_sb[n][:])
                    else:
                        nc.vector.tensor_copy(dest[:], ps[:])

                return h1, h2

            def seq(*fs):
                def f():
                    for g in fs:
                        g()

                return f

            def tr2(b_, k_):
                def f():
                    transpose_v(b_, k_, pp, "pp")
                    transpose_v(b_, k_ + 1, pp, "pp")

                return f

            # batch-0 qb0: interleave its chunks with the projections and
            # transposes of batch 0 stage by stage -- chunk ktc=2*tb,2*tb+1
            # consumes exactly block tb's keys/values, so it is emitted
            # right after them (K and V projected before Q: the chunks
            # need K/V of every block but Q of block 0 only)
            none = lambda: None

            w_loaded = [False]

            def emit_pass():
                st = attn_open(0, 0)
                for tb in range(NTB // B):
                    load(tb)()
                    grp(tb, "k")()
                    grp(tb, "v")()
                    for kt in range(4 * tb, 4 * tb + 4):
                        transpose_v(0, kt, pp, "pp")
                    if tb == 0:
                        grp(0, "q")()  # this block's own queries
                    if tb == NTB // B - 1:
                        # prefetch batch-1's first block before the last chunks
                        load(NTB // B)()
                    attn_chunks(st, range(2 * tb, 2 * tb + 2))
                    if tb > 0:
                        # q of block tb feeds only query-block tb, emitted
                        # after this stage's chunks to unblock ACT sooner
                        grp(tb, "q")()

                # batch-0 qb1..3 carry batch-1 projections + transposes as
                # fillers (one per chunk, consumed from ktc1; next-block hT
                # loads prefetched mid-qb); the previous block's normalize
                # tail rides as a mid-chunk filler
                fin = attn_close(st)

                # remaining blocks: (b, qb, fillers); each block's first QK
                # is hoisted into the PREVIOUS block's last chunk (between
                # its exps and PVs) so ACT rolls straight across boundaries
                blocks = []
                for qb in range(1, NQB):
                    tb = NTB // B + qb - 1  # batch-1 block this qb builds
                    k0 = 4 * (qb - 1)  # batch-1 k-tiles covered by the block
                    q1, q2 = grp_halves(tb, "q")
                    k1, k2 = grp_halves(tb, "k")
                    v1, v2 = grp_halves(tb, "v")
                    nxt = load(tb + 1) if tb + 1 < NTB else none
                    blocks.append((0, qb, [
                        none,
                        q1,
                        q2,
                        k1,
                        k2,
                        v1,
                        seq(v2, nxt),
                        seq(tr2(1, k0), tr2(1, k0 + 2)),
                    ]))
                tb = NTB - 1
                k1, k2 = grp_halves(tb, "k")
                v1, v2 = grp_halves(tb, "v")
                q1, q2 = grp_halves(tb, "q")
                blocks.append((1, 0, [
                    none,
                    k1,
                    k2,
                    v1,
                    seq(v2, tr2(1, 12)),
                    tr2(1, 14),
                    q1,
                    q2,
                ]))
                for qb in range(1, NQB):
                    blocks.append((1, qb, []))

                carry = None  # (state, hoisted first-chunk QKs)
                for i, (bb, qb, fillers) in enumerate(blocks):
                    if carry is None:
                        # first block after qb0: hoist ahead of qb0's close
                        st = attn_open(bb, qb)
                        my_pre = attn_qk(st, 0)
                        fin()
                        mids = []
                    else:
                        st, my_pre = carry
                        mids = [fin]
                    # the NEXT block's open + first QK are emitted at this
                    # block's LAST chunk mid-point (between exps and PVs),
                    # so ACT rolls straight across the boundary
                    box = {}
                    if i + 1 < len(blocks):
                        nbb, nqb, _ = blocks[i + 1]

                        def hoist(nbb=nbb, nqb=nqb, box=box):
                            s = attn_open(nbb, nqb)
                            box["carry"] = (s, attn_qk(s, 0))

                        mids = mids + [none] * (NKT // KTC - 1 - len(mids)) + [hoist]
                    attn_chunks(st, range(NKT // KTC), fillers,
                                mid_fillers=mids, pre_qk=my_pre)
                    fin = attn_close(st)
                    carry = box.get("carry")
                fin()

            for _ in range(reps):
                emit_pass()
    nc.compile()
    return nc


_BUILD_CACHE = {}


def _get_nc(use_mask, use_bias):
    key = (use_mask, use_bias)
    if key not in _BUILD_CACHE:
        _BUILD_CACHE[key] = build(use_mask, use_bias)
    return _BUILD_CACHE[key]


def kernel(hidden_states, attention_mask, Wq, bq, Wk, bk, Wv, bv, _trace=False):
    hidden = np.ascontiguousarray(np.asarray(hidden_states, dtype=np.float32))
    mask = np.asarray(attention_mask, dtype=np.float32).reshape(B, S)
    Wq = np.asarray(Wq, dtype=np.float32)
    Wk = np.asarray(Wk, dtype=np.float32)
    Wv = np.asarray(Wv, dtype=np.float32)
    bq = np.asarray(bq, dtype=np.float32)
    bk = np.asarray(bk, dtype=np.float32)
    bv = np.asarray(bv, dtype=np.float32)

    use_mask = bool(np.any(mask != 0.0))
    use_bias = bool(np.any(bq != 0.0) or np.any(bk != 0.0) or np.any(bv != 0.0))
    nc = _get_nc(use_mask, use_bias)

    hT = np.ascontiguousarray(hidden.reshape(T, H).T)  # [H, T]
    in_maps = []
    for c in range(NCORES):
        sl = slice(c * D, (c + 1) * D)
        m = {
            "hT": hT,
            "wq": np.ascontiguousarray(Wq[:, sl]),
            "wk": np.ascontiguousarray(Wk[:, sl]),
            "wv": np.ascontiguousarray(Wv[:, sl]),
        }
        if use_bias:
            m["bq"] = np.ascontiguousarray(bq[sl].reshape(D, 1))
            m["bk"] = np.ascontiguousarray(bk[sl].reshape(D, 1))
            m["bv"] = np.ascontiguousarray(bv[sl].reshape(D, 1))
        if use_mask:
            # [B, S] -> [128, B, NKT]: partition p holds key kt*128+p
            m["mask"] = np.ascontiguousarray(
                mask.reshape(B, NKT, 128).transpose(2, 0, 1)
            )
        in_maps.append(m)

    res = run_bass_kernel_spmd(
        nc, in_maps, core_ids=list(range(NCORES)), trace=_trace
    )
    # assemble: core c's [128, T] d-major slice -> rows c*128:(c+1)*128
    full_dT = np.concatenate([res.results[c]["out"] for c in range(NCORES)], axis=0)
    out = np.ascontiguousarray(full_dT.T).reshape(B, S, H).astype(np.float32)
    if _trace:
        return out, res
    return out



# revision 5
# speedup vs baseline: 1.0352x; 1.0352x over previous
"""BertSelfAttention on 8 Trainium2 NeuronCores (Bass/Tile).

Sharding: tensor-parallel over heads. 16 heads / 8 cores = 2 heads (128
head-dim columns) per core. Each core computes the Q/K/V projections for
its 128 output dims over all 4096 tokens, then attention for its 2 heads
over both batches, producing a [128, 4096] d-major slice of the output.
The host transposes hidden_states once (to bf16), feeds every core the
same [1024, 4096] activation matrix plus its private weight slice, and
reassembles the full [2, 2048, 1024] output from the 8 slices.

Cost-model-driven design (TimelineSim):
  - All matmul operands bf16 (1 cyc/row at any free size; fp8 DoubleRow
    would halve PE time but its ~5% proportional operand error fails the
    2e-2 gate).  PE busy ~150us: Q/K proj 65.5K + V^T proj 32.8K +
    QK 131K + PV 131K cycles.
  - V is projected DIRECTLY TRANSPOSED: stationary hT-tile [c,128 tok],
    moving Wv [c,128 dims] -> psum [tok, dim], so no PE transposes.
    The [128,130] vtm tiles get a ones column per head; the PV matmul
    then also produces the softmax denominator in psum row 64.
  - exp on ACT costs free*0.833ns + 404ns/inst (PSUM input). Scores are
    chunked [3,3,3,3,2,2] key-tiles per head (6 insts instead of 8),
    via two 3-bank psum tags A/B in strict alternation: ACT 148us busy.
  - PSUM budget (8 banks): A(3) + B(3) + ctx(1) + proj(1).  ctx is
    single-buffered: PV runs as a deferred burst per (qb, head) after
    all 6 chunks, so ctx lifetime is short and bursts serialize safely.
  - softmax normalize: DVE reciprocal of psum row 64, GPSIMD
    partition_broadcast (806ns, idle engine) replaces the PE broadcast
    matmul + DVE staging copy, then one DVE multiply.
  - No PE warmup: TimelineSim's pstate ramp depends only on sim time
    (full speed after 3us), and the first matmul lands ~4us in.
  - Emission order is the per-engine schedule: projections, V^T tiles,
    PV-burst halves and finish chains are woven as filler thunks
    between score chunks, demand-driven (ensure_*) so any weave is
    correct and only timing varies.
"""

from collections import deque

import numpy as np

import concourse.tile as tile
from concourse import bacc, mybir
from concourse.bass_utils import run_bass_kernel_spmd

# Problem shape (hardcoded; harness contract)
B, S, H = 2, 2048, 1024
NUM_HEADS, DH = 16, 64
NCORES = 8
T = B * S                 # 4096 tokens total
D = H // NCORES           # 128 output dims per core (2 heads)
KC = H // 128             # 8 contraction chunks for projections
QB = 512                  # query-block width (one psum bank)
NQB = S // QB             # 4 query blocks per batch
NTB = T // QB             # 8 projection token-blocks
NKT = S // 128            # 16 key tiles per batch
SCALE = 1.0 / float(np.sqrt(DH))

F32 = mybir.dt.float32
BF16 = mybir.dt.bfloat16
EXP = mybir.ActivationFunctionType.Exp

# per-head chunk pattern: (kt0, nkt), psum tags alternate A,B,A,B,A,B
CHUNKS = [(0, 3), (3, 3), (6, 3), (9, 3), (12, 2), (14, 2)]


def build(use_mask: bool, use_bias: bool):
    nc = bacc.Bacc("TRN2", target_bir_lowering=False)

    hT = nc.dram_tensor("hT", [H, T], BF16, kind="ExternalInput")
    wq = nc.dram_tensor("wq", [H, D], BF16, kind="ExternalInput")
    wk = nc.dram_tensor("wk", [H, D], BF16, kind="ExternalInput")
    wv = nc.dram_tensor("wv", [H, D], BF16, kind="ExternalInput")
    if use_bias:
        bq = nc.dram_tensor("bq", [D, 1], F32, kind="ExternalInput")
        bk = nc.dram_tensor("bk", [D, 1], F32, kind="ExternalInput")
        bv = nc.dram_tensor("bv", [D, 1], F32, kind="ExternalInput")
    if use_mask:
        # host pre-transposes to [128, B, NKT] so the DMA is contiguous
        mask = nc.dram_tensor("mask", [128, B, NKT], F32, kind="ExternalInput")
    out = nc.dram_tensor("out", [D, T], F32, kind="ExternalOutput")

    hT_r = hT.rearrange("(kc p) t -> p kc t", p=128)
    w_dram = {"q": wq, "k": wk, "v": wv}

    with tile.TileContext(nc) as tc:
        with (
            tc.tile_pool(name="consts", bufs=1) as consts,
            tc.tile_pool(name="qkv", bufs=1) as qkvp,
            tc.tile_pool(name="ht", bufs=8) as htp,
            tc.tile_pool(name="vtm", bufs=2) as vtmp,
            tc.tile_pool(name="et", bufs=6) as etp,
            tc.tile_pool(name="small", bufs=3) as smallp,
            tc.tile_pool(name="psA", bufs=1, space="PSUM") as psA,
            tc.tile_pool(name="psB", bufs=1, space="PSUM") as psB,
            tc.tile_pool(name="ctxp", bufs=1, space="PSUM") as ctxp,
            tc.tile_pool(name="pp", bufs=1, space="PSUM") as pp,
        ):
            # ---- weights / constants ----
            w_sb = {}
            w_loaded = set()

            def load_w(name):
                if name in w_loaded:
                    return
                w_loaded.add(name)
                w_sb[name] = consts.tile([128, KC, D], BF16, tag=f"w{name}", name=f"w{name}")
                nc.sync.dma_start(
                    out=w_sb[name][:],
                    in_=w_dram[name].rearrange("(kc p) d -> p kc d", p=128),
                )

            b_sb = {}
            if use_bias:
                for name, bt in (("q", bq), ("k", bk), ("v", bv)):
                    b_t = consts.tile([128, 1], F32, tag=f"b{name}", name=f"b{name}")
                    nc.sync.dma_start(out=b_t[:], in_=bt[:])
                    b_sb[name] = b_t
            if use_mask:
                mask_sb = consts.tile([128, B, NKT], F32, tag="mask", name="mask")
                nc.sync.dma_start(out=mask_sb[:], in_=mask[:])

            # per-block Q/K tiles (d-major, partitions = 2 heads x 64 dh)
            Qts = [qkvp.tile([128, QB], BF16, tag=f"Qd{i}", name=f"Qd{i}") for i in range(NTB)]
            Kts = [qkvp.tile([128, QB], BF16, tag=f"Kd{i}", name=f"Kd{i}") for i in range(NTB)]

            hts = {}

            def ensure_ht(tb):
                if tb in hts:
                    return
                t0 = tb * QB
                ht_t = htp.tile([128, KC, QB], BF16, tag="ht", name="ht")
                nc.sync.dma_start(out=ht_t[:], in_=hT_r[:, :, t0 : t0 + QB])
                hts[tb] = ht_t

            proj_done = set()

            def ensure_proj(tb, name):
                """Q/K projection for block tb: stationary W [c,128d],
                moving hT [c,512t] -> psum [128d, 512t] -> bf16 sbuf."""
                if (tb, name) in proj_done:
                    return
                proj_done.add((tb, name))
                ensure_ht(tb)
                load_w(name)
                ps = pp.tile([128, QB], F32, tag="pp", name="pps")
                for kc in range(KC):
                    nc.tensor.matmul(
                        ps[:],
                        w_sb[name][:, kc, :],
                        hts[tb][:, kc, :],
                        start=(kc == 0),
                        stop=(kc == KC - 1),
                    )
                dest = {"q": Qts, "k": Kts}[name][tb]
                if use_bias:
                    nc.vector.tensor_scalar_add(dest[:], ps[:], b_sb[name][:])
                else:
                    nc.vector.tensor_copy(dest[:], ps[:])

            # V^T tiles per (b, kt): [128 keys, h*65 + (d | ones)]
            vtms = {}

            def ensure_vt(b, kt):
                if (b, kt) in vtms:
                    return
                g0 = b * S + kt * 128
                tb, off = divmod(g0, QB)
                ensure_ht(tb)
                load_w("v")
                ps = pp.tile([128, QB], F32, tag="pp", name="pps")
                for kc in range(KC):
                    nc.tensor.matmul(
                        ps[:, 0:128],
                        hts[tb][:, kc, off : off + 128],
                        w_sb["v"][:, kc, :],
                        start=(kc == 0),
                        stop=(kc == KC - 1),
                    )
                vt = vtmp.tile([128, 130], BF16, tag=f"vtm{kt}", name=f"vtm{kt}")
                nc.gpsimd.memset(
                    vt[:, 64::65].rearrange("p (a o) -> p a o", o=1), 1.0
                )
                src = ps[:, 0:128].rearrange("p (g c) -> p g c", g=2)
                if use_bias:
                    nc.vector.tensor_scalar_add(
                        vt[:].rearrange("p (g c) -> p g c", g=2)[:, :, 0:64],
                        src,
                        b_sb["v"][:],
                    )
                else:
                    nc.vector.tensor_copy(
                        vt[:].rearrange("p (g c) -> p g c", g=2)[:, :, 0:64], src
                    )
                vtms[(b, kt)] = vt

            # ---- attention streams ----
            def stream(b, qb, h, mids):
                """Emit QK + exp for the 6 chunks of one (batch, query
                block, head). mids[ci] thunks are emitted after chunk
                ci's exp (filler weave). Returns et list for the PV
                burst."""
                qtb = b * NQB + qb
                ensure_proj(qtb, "q")
                ets = []
                for ci, (k0, nk) in enumerate(CHUNKS):
                    pool = psA if ci % 2 == 0 else psB
                    tag = "A" if ci % 2 == 0 else "B"
                    sps = pool.tile([128, 3, QB], F32, tag=tag, name=f"sps{tag}")
                    for j in range(nk):
                        kt = k0 + j
                        tbi = b * NQB + kt // 4
                        off = (kt % 4) * 128
                        ensure_proj(tbi, "k")
                        nc.tensor.matmul(
                            sps[:, j, :],
                            Kts[tbi][h * 64 : (h + 1) * 64, off : off + 128],
                            Qts[qtb][h * 64 : (h + 1) * 64, :],
                            start=True,
                            stop=True,
                        )
                    et = etp.tile([128, 3, QB], BF16, tag=f"et{tag}", name=f"et{tag}")
                    if use_mask:
                        for j in range(nk):
                            kt = k0 + j
                            nc.scalar.activation(
                                et[:, j, :],
                                sps[:, j, :],
                                EXP,
                                bias=mask_sb[:, b, kt : kt + 1],
                                scale=SCALE,
                            )
                    else:
                        nc.scalar.activation(
                            et[:, 0:nk, :], sps[:, 0:nk, :], EXP, scale=SCALE
                        )
                    ets.append((et, k0, nk))
                    for t in mids[ci]:
                        t()
                return ets

            def make_burst(b, qb, h, ets):
                """Two PV quanta; q2 carries the normalize/store tail."""
                q0 = b * S + qb * QB
                box = {}

                def pv(lo, hi):
                    if "ctx" not in box:
                        box["ctx"] = ctxp.tile([128, QB], F32, tag="ctx", name="ctx")
                    ctx = box["ctx"]
                    for et, k0, nk in ets:
                        for j in range(nk):
                            kt = k0 + j
                            if not (lo <= kt < hi):
                                continue
                            ensure_vt(b, kt)
                            nc.tensor.matmul(
                                ctx[0:65, :],
                                vtms[(b, kt)][:, h * 65 : (h + 1) * 65],
                                et[:, j, :],
                                start=(kt == 0),
                                stop=(kt == NKT - 1),
                            )

                def q1():
                    pv(0, 8)

                def q2():
                    pv(8, NKT)
                    ctx = box["ctx"]
                    rec = smallp.tile([65, QB], F32, tag="rec", name="rec")
                    with nc.allow_low_precision(reason="softmax denom recip"):
                        nc.vector.reciprocal(rec[64:65, :], ctx[64:65, :])
                    bc = smallp.tile([64, QB], F32, tag="bc", name="bc")
                    nc.gpsimd.partition_broadcast(bc[:], rec[64:65, :])
                    ot = smallp.tile([64, QB], F32, tag="ot", name="ot")
                    nc.vector.tensor_mul(ot[:], ctx[0:64, :], bc[:])
                    nc.sync.dma_start(
                        out=out[h * 64 : (h + 1) * 64, q0 : q0 + QB], in_=ot[:]
                    )

                return [q1, q2]

            # ---- global weave ----
            load_w("k")
            load_w("q")
            ensure_ht(0)
            ensure_proj(0, "k")
            ensure_proj(0, "q")
            ensure_ht(1)
            load_w("v")
            ensure_ht(2)

            def TH(tb):
                return lambda: ensure_ht(tb)

            def TK(tb):
                return lambda: ensure_proj(tb, "k")

            def TV(b, kt):
                return lambda: ensure_vt(b, kt)

            PREFETCH = {
                (0, 0, 0): [TH(3)],
                (0, 0, 1): [TV(0, k) for k in range(0, 4)],
                (0, 1, 0): [TV(0, k) for k in range(4, 8)],
                (0, 1, 1): [TV(0, k) for k in range(8, 12)],
                (0, 2, 0): [TV(0, k) for k in range(12, 16)],
                (0, 2, 1): [TH(4), TK(4), TH(5)],
                (0, 3, 0): [TK(5), TH(6)],
                (0, 3, 1): [TK(6), TH(7), TK(7)],
                (1, 0, 0): [TV(1, k) for k in range(0, 4)],
                (1, 0, 1): [TV(1, k) for k in range(4, 8)],
                (1, 1, 0): [TV(1, k) for k in range(8, 12)],
                (1, 1, 1): [TV(1, k) for k in range(12, 16)],
            }

            pending = []
            for b in range(B):
                for qb in range(NQB):
                    for h in (0, 1):
                        mids = [[] for _ in range(len(CHUNKS))]
                        if pending:
                            mids[0].append(pending[0])
                            mids[1].append(pending[1])
                            pending = []
                        for i, t in enumerate(PREFETCH.get((b, qb, h), [])):
                            mids[2 + (i % 4)].append(t)
                        ets = stream(b, qb, h, mids)
                        pending = make_burst(b, qb, h, ets)
            for t in pending:
                t()

    nc.compile()
    return nc


_BUILD_CACHE = {}


def _get_nc(use_mask, use_bias):
    key = (use_mask, use_bias)
    if key not in _BUILD_CACHE:
        _BUILD_CACHE[key] = build(use_mask, use_bias)
    return _BUILD_CACHE[key]


def kernel(hidden_states, attention_mask, Wq, bq, Wk, bk, Wv, bv, _trace=False):
    import ml_dtypes

    hidden = np.ascontiguousarray(np.asarray(hidden_states, dtype=np.float32))
    mask = np.asarray(attention_mask, dtype=np.float32).reshape(B, S)
    Wq = np.asarray(Wq, dtype=np.float32)
    Wk = np.asarray(Wk, dtype=np.float32)
    Wv = np.asarray(Wv, dtype=np.float32)
    bq = np.asarray(bq, dtype=np.float32)
    bk = np.asarray(bk, dtype=np.float32)
    bv = np.asarray(bv, dtype=np.float32)

    use_mask = bool(np.any(mask != 0.0))
    use_bias = bool(np.any(bq != 0.0) or np.any(bk != 0.0) or np.any(bv != 0.0))
    nc = _get_nc(use_mask, use_bias)

    bf = ml_dtypes.bfloat16
    hT = np.ascontiguousarray(hidden.reshape(T, H).T).astype(bf)  # [H, T]
    in_maps = []
    for c in range(NCORES):
        sl = slice(c * D, (c + 1) * D)
        m = {
            "hT": hT,
            "wq": np.ascontiguousarray(Wq[:, sl]).astype(bf),
            "wk": np.ascontiguousarray(Wk[:, sl]).astype(bf),
            "wv": np.ascontiguousarray(Wv[:, sl]).astype(bf),
        }
        if use_bias:
            m["bq"] = np.ascontiguousarray(bq[sl].reshape(D, 1))
            m["bk"] = np.ascontiguousarray(bk[sl].reshape(D, 1))
            m["bv"] = np.ascontiguousarray(bv[sl].reshape(D, 1))
        if use_mask:
            # [B, S] -> [128, B, NKT]: partition p holds key kt*128+p
            m["mask"] = np.ascontiguousarray(
                mask.reshape(B, NKT, 128).transpose(2, 0, 1)
            )
        in_maps.append(m)

    res = run_bass_kernel_spmd(
        nc, in_maps, core_ids=list(range(NCORES)), trace=_trace
    )
    # assemble: core c's [128, T] d-major slice -> rows c*128:(c+1)*128
    full_dT = np.concatenate([res.results[c]["out"] for c in range(NCORES)], axis=0)
    out = np.ascontiguousarray(full_dT.T).reshape(B, S, H).astype(np.float32)
    if _trace:
        return out, res
    return out


# revision 8
# speedup vs baseline: 1.1110x; 1.0732x over previous
"""BertSelfAttention on 8 Trainium2 NeuronCores (Bass/Tile).

Sharding: tensor-parallel over heads. 16 heads / 8 cores = 2 heads (128
head-dim columns) per core. Each core computes the Q/K/V projections for
its 128 output dims over all 4096 tokens, then attention for its 2 heads
over both batches, producing a [128, 4096] d-major slice of the output.
The host transposes hidden_states once (to bf16), feeds every core the
same [1024, 4096] activation matrix plus its private weight slice, and
reassembles the full [2, 2048, 1024] output from the 8 slices.

Cost-model-driven design (TimelineSim):
  - All matmul operands bf16 (1 cyc/row at any free size; fp8 DoubleRow
    would halve PE time but its ~5% proportional operand error fails the
    2e-2 gate).  PE busy ~150us: Q/K proj 65.5K + V^T proj 32.8K +
    QK 131K + PV 131K cycles.
  - V is projected DIRECTLY TRANSPOSED: stationary hT-tile [c,128 tok],
    moving Wv [c,128 dims] -> psum [tok, dim], so no PE transposes.
    The [128,130] vtm tiles get a ones column per head; the PV matmul
    then also produces the softmax denominator in psum row 64.
  - exp on ACT costs free*0.833ns + 404ns/inst (PSUM input). Scores are
    chunked [3,3,3,3,2,2] key-tiles per head (6 insts instead of 8),
    via two 3-bank psum tags A/B in strict alternation: ACT 148us busy.
  - PSUM budget (8 banks): A(3) + B(3) + ctx(1) + proj(1).  ctx is
    single-buffered: PV runs as a deferred burst per (qb, head) after
    all 6 chunks, so ctx lifetime is short and bursts serialize safely.
  - softmax normalize: DVE reciprocal of psum row 64, GPSIMD
    partition_broadcast (806ns, idle engine) replaces the PE broadcast
    matmul + DVE staging copy, then one DVE multiply.
  - No PE warmup: TimelineSim's pstate ramp depends only on sim time
    (full speed after 3us), and the first matmul lands ~4us in.
  - Emission order is the per-engine schedule: projections, V^T tiles,
    PV-burst halves and finish chains are woven as filler thunks
    between score chunks, demand-driven (ensure_*) so any weave is
    correct and only timing varies.
"""

from collections import deque

import numpy as np

import concourse.tile as tile
from concourse import bacc, mybir
from concourse.bass_utils import run_bass_kernel_spmd

# Problem shape (hardcoded; harness contract)
B, S, H = 2, 2048, 1024
NUM_HEADS, DH = 16, 64
NCORES = 8
T = B * S                 # 4096 tokens total
D = H // NCORES           # 128 output dims per core (2 heads)
KC = H // 128             # 8 contraction chunks for projections
QB = 512                  # query-block width (one psum bank)
NQB = S // QB             # 4 query blocks per batch
NTB = T // QB             # 8 projection token-blocks
NKT = S // 128            # 16 key tiles per batch
SCALE = 1.0 / float(np.sqrt(DH))

F32 = mybir.dt.float32
BF16 = mybir.dt.bfloat16
EXP = mybir.ActivationFunctionType.Exp

REGIONS = []  # (label, "I-<n>") probes for trace attribution

# per-head chunk pattern: (kt0, nkt), psum tags alternate A,B,A,B,A,B
CHUNKS = [(0, 3), (3, 3), (6, 3), (9, 3), (12, 2), (14, 2)]


def build(use_mask: bool, use_bias: bool):
    nc = bacc.Bacc("TRN2", target_bir_lowering=False)
    REGIONS.clear()

    def probe(label):
        REGIONS.append((label, nc.get_next_instruction_name()))

    hT = nc.dram_tensor("hT", [H, T], BF16, kind="ExternalInput")
    wq = nc.dram_tensor("wq", [H, D], BF16, kind="ExternalInput")
    wk = nc.dram_tensor("wk", [H, D], BF16, kind="ExternalInput")
    wv = nc.dram_tensor("wv", [H, D], BF16, kind="ExternalInput")
    if use_bias:
        bq = nc.dram_tensor("bq", [D, 1], F32, kind="ExternalInput")
        bk = nc.dram_tensor("bk", [D, 1], F32, kind="ExternalInput")
        bv = nc.dram_tensor("bv", [D, 1], F32, kind="ExternalInput")
    if use_mask:
        # host pre-transposes to [128, B, NKT] so the DMA is contiguous
        mask = nc.dram_tensor("mask", [128, B, NKT], F32, kind="ExternalInput")
    out = nc.dram_tensor("out", [D, T], F32, kind="ExternalOutput")

    hT_r = hT.rearrange("(kc p) t -> p kc t", p=128)
    w_dram = {"q": wq, "k": wk, "v": wv}

    with tile.TileContext(nc) as tc:
        with (
            tc.tile_pool(name="consts", bufs=1) as consts,
            tc.tile_pool(name="qkv", bufs=1) as qkvp,
            tc.tile_pool(name="ht", bufs=8) as htp,
            tc.tile_pool(name="vtm", bufs=2) as vtmp,
            tc.tile_pool(name="et", bufs=6) as etp,
            tc.tile_pool(name="small", bufs=3) as smallp,
            tc.tile_pool(name="psA", bufs=1, space="PSUM") as psA,
            tc.tile_pool(name="psB", bufs=1, space="PSUM") as psB,
            tc.tile_pool(name="ctxp", bufs=1, space="PSUM") as ctxp,
            tc.tile_pool(name="pp", bufs=1, space="PSUM") as pp,
        ):
            # ---- weights / constants ----
            w_sb = {}
            w_loaded = set()

            def load_w(name):
                if name in w_loaded:
                    return
                w_loaded.add(name)
                w_sb[name] = consts.tile([128, KC, D], BF16, tag=f"w{name}", name=f"w{name}")
                nc.sync.dma_start(
                    out=w_sb[name][:],
                    in_=w_dram[name].rearrange("(kc p) d -> p kc d", p=128),
                )

            b_sb = {}
            if use_bias:
                for name, bt in (("q", bq), ("k", bk), ("v", bv)):
                    b_t = consts.tile([128, 1], F32, tag=f"b{name}", name=f"b{name}")
                    nc.sync.dma_start(out=b_t[:], in_=bt[:])
                    b_sb[name] = b_t
            if use_mask:
                mask_sb = consts.tile([128, B, NKT], F32, tag="mask", name="mask")
                nc.sync.dma_start(out=mask_sb[:], in_=mask[:])

            # per-block Q/K tiles (d-major, partitions = 2 heads x 64 dh)
            Qts = [qkvp.tile([128, QB], BF16, tag=f"Qd{i}", name=f"Qd{i}") for i in range(NTB)]
            Kts = [qkvp.tile([128, QB], BF16, tag=f"Kd{i}", name=f"Kd{i}") for i in range(NTB)]

            hts = {}

            def ensure_ht(tb):
                if tb in hts:
                    return
                t0 = tb * QB
                ht_t = htp.tile([128, KC, QB], BF16, tag="ht", name="ht")
                nc.sync.dma_start(out=ht_t[:], in_=hT_r[:, :, t0 : t0 + QB])
                hts[tb] = ht_t

            proj_done = set()

            def ensure_proj(tb, name):
                """Q/K projection for block tb: stationary W [c,128d],
                moving hT [c,512t] -> psum [128d, 512t] -> bf16 sbuf."""
                if (tb, name) in proj_done:
                    return
                proj_done.add((tb, name))
                ensure_ht(tb)
                load_w(name)
                probe(f"proj_{name}{tb}")
                ps = pp.tile([128, QB], F32, tag="pp", name="pps")
                for kc in range(KC):
                    nc.tensor.matmul(
                        ps[:],
                        w_sb[name][:, kc, :],
                        hts[tb][:, kc, :],
                        start=(kc == 0),
                        stop=(kc == KC - 1),
                    )
                dest = {"q": Qts, "k": Kts}[name][tb]
                if use_bias:
                    nc.vector.tensor_scalar_add(dest[:], ps[:], b_sb[name][:])
                else:
                    nc.vector.tensor_copy(dest[:], ps[:])

            # V^T tiles per (b, kt): [128 keys, h*65 + (d | ones)]
            vtms = {}

            def ensure_vt(b, kt, _batch=None):
                if (b, kt) in vtms:
                    return
                kts = [kt] if _batch is None else [k for k in _batch if (b, k) not in vtms]
                for k in kts:
                    ensure_ht((b * S + k * 128) // QB)
                load_w("v")
                probe(f"vt_{b}_{kt}")
                ps = pp.tile([128, QB], F32, tag="pp", name="pps")
                # up to 4 V^T projections share the bank at 128-col offsets,
                # so only one copy-out WAR stall per group
                for i, k in enumerate(kts):
                    g0 = b * S + k * 128
                    tb, off = divmod(g0, QB)
                    for kc in range(KC):
                        nc.tensor.matmul(
                            ps[:, 128 * i : 128 * (i + 1)],
                            hts[tb][:, kc, off : off + 128],
                            w_sb["v"][:, kc, :],
                            start=(kc == 0),
                            stop=(kc == KC - 1),
                        )
                for i, k in enumerate(kts):
                    vt = vtmp.tile([128, 130], BF16, tag=f"vtm{k}", name=f"vtm{k}")
                    nc.gpsimd.memset(
                        vt[:, 64::65].rearrange("p (a o) -> p a o", o=1), 1.0
                    )
                    srcp = ps[:, 128 * i : 128 * (i + 1)].rearrange(
                        "p (g c) -> p g c", g=2
                    )
                    if use_bias:
                        nc.vector.tensor_scalar_add(
                            vt[:].rearrange("p (g c) -> p g c", g=2)[:, :, 0:64],
                            srcp,
                            b_sb["v"][:],
                        )
                    else:
                        nc.vector.tensor_copy(
                            vt[:].rearrange("p (g c) -> p g c", g=2)[:, :, 0:64],
                            srcp,
                        )
                    vtms[(b, k)] = vt

            # ---- attention streams ----
            def stream(b, qb, h, mids, ets):
                """Emit QK + exp for the 6 chunks of one (batch, query
                block, head). mids[ci] thunks are emitted after chunk
                ci's exp (filler weave); et tiles are appended to `ets`
                as chunks are emitted (the PV burst reads it live)."""
                qtb = b * NQB + qb
                ensure_proj(qtb, "q")
                for ci, (k0, nk) in enumerate(CHUNKS):
                    probe(f"qk_{b}{qb}{h}_c{ci}")
                    pool = psA if ci % 2 == 0 else psB
                    tag = "A" if ci % 2 == 0 else "B"
                    sps = pool.tile([128, 3, QB], F32, tag=tag, name=f"sps{tag}")
                    for j in range(nk):
                        kt = k0 + j
                        tbi = b * NQB + kt // 4
                        off = (kt % 4) * 128
                        ensure_proj(tbi, "k")
                        nc.tensor.matmul(
                            sps[:, j, :],
                            Kts[tbi][h * 64 : (h + 1) * 64, off : off + 128],
                            Qts[qtb][h * 64 : (h + 1) * 64, :],
                            start=True,
                            stop=True,
                        )
                    et = etp.tile([128, 3, QB], BF16, tag=f"et{tag}", name=f"et{tag}")
                    if use_mask:
                        for j in range(nk):
                            kt = k0 + j
                            nc.scalar.activation(
                                et[:, j, :],
                                sps[:, j, :],
                                EXP,
                                bias=mask_sb[:, b, kt : kt + 1],
                                scale=SCALE,
                            )
                    else:
                        nc.scalar.activation(
                            et[:, 0:nk, :], sps[:, 0:nk, :], EXP, scale=SCALE
                        )
                    ets.append((et, k0, nk))
                    probe(f"mid_{b}{qb}{h}_c{ci}")
                    for t in mids[ci]:
                        t()

            def make_burst(b, qb, h, ets):
                """Two PV quanta; q2 carries the normalize/store tail."""
                q0 = b * S + qb * QB
                box = {}

                def pv(lo, hi):
                    if "ctx" not in box:
                        box["ctx"] = ctxp.tile([128, QB], F32, tag="ctx", name="ctx")
                    ctx = box["ctx"]
                    for et, k0, nk in ets:
                        for j in range(nk):
                            kt = k0 + j
                            if not (lo <= kt < hi):
                                continue
                            ensure_vt(b, kt)
                            nc.tensor.matmul(
                                ctx[0:65, :],
                                vtms[(b, kt)][:, h * 65 : (h + 1) * 65],
                                et[:, j, :],
                                start=(kt == 0),
                                stop=(kt == NKT - 1),
                            )

                def q1():
                    probe(f"pv1_{b}{qb}{h}")
                    pv(0, 8)

                def q2():
                    probe(f"pv2_{b}{qb}{h}")
                    pv(8, NKT)
                    ctx = box["ctx"]
                    rec = smallp.tile([65, QB], F32, tag="rec", name="rec")
                    with nc.allow_low_precision(reason="softmax denom recip"):
                        nc.vector.reciprocal(rec[64:65, :], ctx[64:65, :])
                    bc = smallp.tile([64, QB], F32, tag="bc", name="bc")
                    nc.gpsimd.partition_broadcast(bc[:], rec[64:65, :])
                    ot = smallp.tile([64, QB], F32, tag="ot", name="ot")
                    nc.vector.tensor_mul(ot[:], ctx[0:64, :], bc[:])
                    nc.sync.dma_start(
                        out=out[h * 64 : (h + 1) * 64, q0 : q0 + QB], in_=ot[:]
                    )

                return [q1, q2]

            # ---- global weave ----
            # PE warmup: the cost model rates matmuls at SEQ-dispatch time,
            # and everything dispatched before ~3us wall clock runs at the
            # low/mid pstate. Burn the early dispatch slots on dummy
            # matmuls gated only on a memset (no DMA), so real matmuls
            # dispatch past the ramp.
            warm = consts.tile([128, 64], BF16, tag="warm", name="warm")
            nc.vector.memset(warm[:], 1.0)
            for _ in range(48):
                wps = pp.tile([128, QB], F32, tag="pp", name="pps")
                nc.tensor.matmul(
                    wps[0:64, 0:64], warm[:, :], warm[:, :], start=True, stop=True
                )
            load_w("k")
            ensure_ht(0)
            load_w("q")
            ensure_proj(0, "k")
            ensure_proj(0, "q")
            ensure_ht(1)
            load_w("v")
            ensure_ht(2)

            def TH(tb):
                return lambda: ensure_ht(tb)

            def TK(tb):
                return lambda: ensure_proj(tb, "k")

            def TQ(tb):
                return lambda: ensure_proj(tb, "q")

            def TV4(b, k0):
                return lambda: ensure_vt(b, k0, _batch=range(k0, k0 + 4))

            # mid-slot prefetch map: (b, qb, h) -> {slot: [thunks]}
            PREFETCH = {
                (0, 0, 0): {0: [TK(1)], 2: [TH(3), TK(2)], 3: [TV4(0, 0)],
                            4: [TV4(0, 4), TK(3)], 5: [TV4(0, 8)]},
                (0, 0, 1): {0: [TV4(0, 12)], 2: [TQ(1)]},
                (0, 1, 0): {4: [TH(4)]},
                (0, 1, 1): {2: [TQ(2)], 4: [TH(5)]},
                (0, 2, 0): {4: [TH(6)]},
                (0, 2, 1): {2: [TK(4)], 4: [TH(7)], 5: [TQ(3)]},
                (0, 3, 0): {2: [TK(5)], 4: [TK(6)], 5: [TV4(1, 0)]},
                (0, 3, 1): {2: [TK(7)], 4: [TQ(4)], 5: [TV4(1, 4)]},
                (1, 0, 0): {3: [TV4(1, 8)], 5: [TV4(1, 12)]},
                (1, 0, 1): {2: [TQ(5)]},
                (1, 1, 1): {2: [TQ(6)]},
                (1, 2, 1): {2: [TQ(7)]},
            }

            pending = []
            streams = [(b, qb, h) for b in range(B) for qb in range(NQB)
                       for h in (0, 1)]
            for si, (b, qb, h) in enumerate(streams):
                mids = [[] for _ in range(len(CHUNKS))]
                if pending:
                    mids[1].append(pending[0])
                    mids[3].append(pending[1])
                    pending = []
                for slot, ts_ in PREFETCH.get((b, qb, h), {}).items():
                    mids[slot].extend(ts_)
                ets = []
                burst = make_burst(b, qb, h, ets)
                if si == len(streams) - 1:
                    # final stream: drain its own PV early to shorten the tail
                    mids[4].append(burst[0])
                    stream(b, qb, h, mids, ets)
                    burst[1]()
                else:
                    stream(b, qb, h, mids, ets)
                    pending = burst

    nc.compile()
    return nc


_BUILD_CACHE = {}


def _get_nc(use_mask, use_bias):
    key = (use_mask, use_bias)
    if key not in _BUILD_CACHE:
        _BUILD_CACHE[key] = build(use_mask, use_bias)
    return _BUILD_CACHE[key]


def kernel(hidden_states, attention_mask, Wq, bq, Wk, bk, Wv, bv, _trace=False):
    import ml_dtypes

    hidden = np.ascontiguousarray(np.asarray(hidden_states, dtype=np.float32))
    mask = np.asarray(attention_mask, dtype=np.float32).reshape(B, S)
    Wq = np.asarray(Wq, dtype=np.float32)
    Wk = np.asarray(Wk, dtype=np.float32)
    Wv = np.asarray(Wv, dtype=np.float32)
    bq = np.asarray(bq, dtype=np.float32)
    bk = np.asarray(bk, dtype=np.float32)
    bv = np.asarray(bv, dtype=np.float32)

    use_mask = bool(np.any(mask != 0.0))
    use_bias = bool(np.any(bq != 0.0) or np.any(bk != 0.0) or np.any(bv != 0.0))
    nc = _get_nc(use_mask, use_bias)

    bf = ml_dtypes.bfloat16
    hT = np.ascontiguousarray(hidden.reshape(T, H).T).astype(bf)  # [H, T]
    in_maps = []
    for c in range(NCORES):
        sl = slice(c * D, (c + 1) * D)
        m = {
            "hT": hT,
            "wq": np.ascontiguousarray(Wq[:, sl]).astype(bf),
            "wk": np.ascontiguousarray(Wk[:, sl]).astype(bf),
            "wv": np.ascontiguousarray(Wv[:, sl]).astype(bf),
        }
        if use_bias:
            m["bq"] = np.ascontiguousarray(bq[sl].reshape(D, 1))
            m["bk"] = np.ascontiguousarray(bk[sl].reshape(D, 1))
            m["bv"] = np.ascontiguousarray(bv[sl].reshape(D, 1))
        if use_mask:
            # [B, S] -> [128, B, NKT]: partition p holds key kt*128+p
            m["mask"] = np.ascontiguousarray(
                mask.reshape(B, NKT, 128).transpose(2, 0, 1)
            )
        in_maps.append(m)

    res = run_bass_kernel_spmd(
        nc, in_maps, core_ids=list(range(NCORES)), trace=_trace
    )
    # assemble: core c's [128, T] d-major slice -> rows c*128:(c+1)*128
    full_dT = np.concatenate([res.results[c]["out"] for c in range(NCORES)], axis=0)
    out = np.ascontiguousarray(full_dT.T).reshape(B, S, H).astype(np.float32)
    if _trace:
        return out, res
    return out
